# revision 1
# baseline (speedup 1.0000x reference)
"""Trainium2 (8 NeuronCores, SPMD) kernel for nn_AntiAliasInterpolation2d:
depthwise 13x13 gaussian blur + 4x nearest subsample on x [8, 64, 512, 512] f32.

Strategy
--------
Pure data parallel: batch dim (8) shards 1:1 across the 8 cores; no
cross-core communication. Per core: x_b [64, 512, 512] -> out_b [64, 128, 128].

The gaussian kernel is exactly separable (rank-1: w2d = g_row g_col^T, taken
from the SVD of the passed weight). Fused blur+subsample per channel is then
    out = Av @ X @ Ah^T
with Av/Ah [128, 512] banded stride-4 conv matrices (edge taps truncated to
match the reference's zero padding). On-chip, per channel:

  1. pass1 on TensorE in float32r (full rate at N=512, no cast of the 64 MB
     input needed): V[128 h_out, 512 w] = sum_j AvT_j^T @ X_j, PSUM f32.
  2. VectorE casts V to bf16 in SBUF.
  3. TensorE transpose (bf16) makes V^T tiles [w, h_out] (pass2 must contract
     over w, and the PE contracts over the partition axis only).
  4. ScalarE copies V^T to SBUF; pass2 on TensorE in bf16:
     O[h_out, w_out] = sum_t (V^T_t)^T @ AhT_t, PSUM f32.
  5. ScalarE copies O into a 4-channel store buffer; stores go on the ACT
     HWDGE ring so they don't serialize against the input loads on SP.

Input loads are batched 4 channels / dma_start (4 MB, 128 partitions), which
saturates the ~320 GB/s per-core HBM read bandwidth; the kernel is DMA-bound
and measures ~203 us/core steady state (= the measured pure-DMA floor).
"""
import sys

sys.path.insert(0, '/opt/trn_rl_repo')

import numpy as np
import ml_dtypes

import concourse.bass as bass
import concourse.mybir as mybir
import concourse.tile as tile
from concourse.bass import ts
from concourse.bass_utils import run_bass_kernel_spmd

F32 = mybir.dt.float32
F32R = mybir.dt.float32r
BF16 = mybir.dt.bfloat16

N_CORES = 8
C = 64
H = W = 512
HO = WO = 128
BATCH = 4            # channels per input dma_start
XBUFS = 4            # input-group prefetch depth


def _fix_multi_waits(nc, limit=1):
    """This walrus build rejects >1 sync wait per instruction (and any wait
    on InstDrain). Hoist excess waits onto injected same-engine NOPs placed
    immediately before the instruction."""
    ctr = [0]
    for f in nc.m.functions:
        for blk in f.blocks:
            il = blk.instructions
            out = []
            changed = False
            for inst in list(il):
                si = getattr(inst, 'sync_info', None)
                waits = list(si.on_wait) if (si and si.on_wait) else []
                lim = 0 if type(inst).__name__ == 'InstDrain' else limit
                if len(waits) > lim:
                    keep, extra = waits[:lim], waits[lim:]
                    for w in extra:
                        ctr[0] += 1
                        nop = mybir.InstNoOp(
                            name=f'I-wsplit-{ctr[0]}', engine=inst.engine,
                            ins=[], outs=[])
                        nop.sync_info = mybir.SyncInfo(on_wait=[w], on_update=[])
                        nc.register_instruction(nop, overwrite=True)
                        out.append(nop)
                    inst.sync_info = mybir.SyncInfo(
                        on_wait=keep,
                        on_update=list(si.on_update) if si.on_update else [])
                    changed = True
                out.append(inst)
            if changed:
                il[:] = out


def _banded_matrix(g13):
    """[128, 512] stride-4 conv matrix with truncated edge taps."""
    A = np.zeros((128, 512), np.float32)
    for o in range(128):
        for k in range(13):
            i = 4 * o + k - 6
            if 0 <= i < 512:
                A[o, i] += np.float32(g13[k])
    return A


def _const_inputs(w2d):
    u, s, vt = np.linalg.svd(w2d.astype(np.float64))
    g_row = u[:, 0] * np.sqrt(s[0])
    g_col = vt[0, :] * np.sqrt(s[0])
    if g_row[6] < 0:
        g_row, g_col = -g_row, -g_col
    AvT = np.ascontiguousarray(_banded_matrix(g_row).T).reshape(4, 128, 128)
    AhT = np.ascontiguousarray(_banded_matrix(g_col).T).reshape(4, 128, 128)
    return {
        'atv': AvT.astype(np.float32),
        'ath': AhT.astype(ml_dtypes.bfloat16),
        'eyeb': np.eye(128, dtype=ml_dtypes.bfloat16),
    }


def build_kernel():
    nc = bass.Bass("TRN2", target_bir_lowering=False, debug=False,
                   num_devices=N_CORES)
    x = nc.declare_dram_parameter('x', [C, H, W], F32, isOutput=False)
    atv = nc.declare_dram_parameter('atv', [4, 128, 128], F32, isOutput=False)
    ath = nc.declare_dram_parameter('ath', [4, 128, 128], BF16, isOutput=False)
    eyeb = nc.declare_dram_parameter('eyeb', [128, 128], BF16, isOutput=False)
    out = nc.declare_dram_parameter('out', [C, HO, WO], F32, isOutput=True)

    ngrp = C // BATCH
    with tile.TileContext(nc) as tc:
        with (
            tc.tile_pool(name='const', bufs=1) as constp,
            tc.tile_pool(name='xp', bufs=XBUFS) as xp,
            tc.tile_pool(name='vp', bufs=3) as vp,
            tc.tile_pool(name='op', bufs=2) as op,
            tc.tile_pool(name='psv', bufs=2, space='PSUM') as psv,
            tc.tile_pool(name='pst', bufs=2, space='PSUM') as pst,
            tc.tile_pool(name='pso', bufs=2, space='PSUM') as pso,
        ):
            atv_t = constp.tile([128, 4, 128], F32R)
            ath_t = constp.tile([128, 4, 128], BF16)
            eye_t = constp.tile([128, 128], BF16)
            nc.sync.dma_start(atv_t[:], atv.rearrange("j p m -> p j m").bitcast(F32R))
            nc.sync.dma_start(ath_t[:], ath.rearrange("j p m -> p j m"))
            nc.sync.dma_start(eye_t[:], eyeb[:])

            for g in range(ngrp):
                xbuf = xp.tile([128, BATCH, 4, 512], F32R, tag='xbuf')
                nc.sync.dma_start(
                    xbuf[:],
                    x[g * BATCH:(g + 1) * BATCH]
                    .rearrange("c (j p) w -> p c j w", p=128)
                    .bitcast(F32R))
                obuf = op.tile([128, BATCH, 128], F32, tag='obuf')
                for ci in range(BATCH):
                    psum_v = psv.tile([128, 512], F32, tag='pv')
                    for j in range(4):
                        nc.tensor.matmul(
                            psum_v[:], atv_t[:, j, :], xbuf[:, ci, j, :],
                            start=(j == 0), stop=(j == 3))

                    vbuf = vp.tile([128, 512], BF16, tag='vbuf')
                    nc.vector.tensor_copy(vbuf[:], psum_v[:])

                    psum_vt = pst.tile([128, 4, 128], BF16, tag='pvt')
                    for t in range(4):
                        nc.tensor.transpose(
                            psum_vt[:, t, :], vbuf[:, ts(t, 128)], eye_t[:])

                    vtbuf = vp.tile([128, 4, 128], BF16, tag='vtbuf')
                    nc.scalar.copy(vtbuf[:], psum_vt[:])

                    psum_o = pso.tile([128, 128], F32, tag='po')
                    for t in range(4):
                        nc.tensor.matmul(
                            psum_o[:], vtbuf[:, t, :], ath_t[:, t, :],
                            start=(t == 0), stop=(t == 3))

                    nc.scalar.copy(obuf[:, ci, :], psum_o[:])

                # output store on the ACT HWDGE ring (doesn't serialize
                # against the input loads on the SP ring)
                nc.scalar.dma_start(
                    out[g * BATCH:(g + 1) * BATCH].rearrange("c p w -> p c w"),
                    obuf[:])

    _fix_multi_waits(nc)
    return nc


_CACHE = {}


def kernel(x, weight):
    x = np.ascontiguousarray(np.asarray(x), dtype=np.float32)
    weight = np.asarray(weight)
    assert x.shape == (8, C, H, W), x.shape

    if 'nc' not in _CACHE:
        _CACHE['nc'] = build_kernel()
    nc = _CACHE['nc']

    consts = _const_inputs(np.asarray(weight[0, 0], dtype=np.float32))
    in_maps = [dict(x=x[b], **consts) for b in range(N_CORES)]
    res = run_bass_kernel_spmd(nc, in_maps, core_ids=list(range(N_CORES)))
    out = np.stack([np.asarray(res.results[b]['out']) for b in range(N_CORES)])
    return out.astype(np.float32)
